# revision 21
# baseline (speedup 1.0000x reference)
"""Trainium2 Bass kernel: 15x15 valid cross-correlation of 4096x4096 (+bias).

Sharding: output rows split across 8 NeuronCores (512 rows/core, 14-row
halo), gathered on the host.

Per core the conv runs as a 2-shift im2col-lite on the TensorEngine: the
K-lanes hold two column-shifted views of a 46-row x slab (partitions
0..45 = shift 0, 46..91 = shift +1 col) and each pass's stationary matrix
maps lane (shift c', row r') to output (col-phase bw in 0..3, row bh in
0..31):

    T_p[c'*46+r', bw*32+bh] = w[r' - bh, 2p + c' - bw]    p = 0..8

so one [92 x 128] x [92 x N] matmul covers 2 kernel columns for a
32-row x 4-col-phase output block (M=128 full): 9 passes/32 rows vs the
banded-Toeplitz 15/114 (147k vs 276k PE cycles/core).

Key DMA tricks (each measured as a cliff on HW):
- shift-0 lanes are only read at even cols (2p+4n), shift-1 at odd, so
  the host packs even cols for lanes 0..45 and odd cols for 46..91 into
  one contiguous [NSETS, 92, 2050] tensor: zero duplication (6 MB/core)
  and the rhs is a uniform stride-2 (8B) AP = full-rate SBUF stream.
- every dma_start must be one contiguous HBM range: a strided read
  chains all descriptors onto a single SDMA engine (~27 GiB/s).
- each HWDGE queue (sync/scalar) only drives ~4 SDMA engines, and a
  queue is FIFO: a store waiting on a drain blocks later loads queued
  behind it.  Loads and stores alternate queues by set parity instead.
- output drains as one contiguous [128, 1021] block per set (psum
  partition = bw*32+bh, col n <-> out col 4n+bw); host re-interleaves.

Weights/x in bf16 (fp32 PSUM accumulate), same numerics as the reference
Toeplitz baseline (~3e-3 rel err).
"""

import numpy as np

H = 4096
W = 4096
KH = 15
KW = 15
OH = H - KH + 1  # 4082
OW = W - KW + 1  # 4082
NCORES = 8
CROWS = 512             # output rows per core
BH = 32                 # output rows per set
BW = 4                  # output col-phases
NSETS = CROWS // BH     # 16
NPASS = 9               # ceil((BW + KW - 1) / 2)
R = BH + KH - 1         # 46 rows per shift-copy
KL = 2 * R              # 92 K-lanes
NCH = (OW + BW - 1) // BW       # 1021 psum cols (4*1021=4084 >= 4082)
TW2 = 2050              # packed half-width (max read idx p+2n = 2048)
XROWS = CROWS + KH - 1  # 526

_CACHE = {}


def _build_program():
    import concourse.tile as tile
    from concourse import bacc, mybir
    from contextlib import ExitStack

    nc = bacc.Bacc("TRN2", target_bir_lowering=False, debug=False,
                   num_devices=NCORES)
    bf16 = mybir.dt.bfloat16
    f32 = mybir.dt.float32
    x_d = nc.dram_tensor("x", [NSETS, KL, TW2], bf16,
                         kind="ExternalInput").ap()
    w_d = nc.dram_tensor("wt", [KL, NPASS * 128], bf16,
                         kind="ExternalInput").ap()
    b_d = nc.dram_tensor("bias", [128, 1], f32, kind="ExternalInput").ap()
    o_d = nc.dram_tensor("out", [NSETS, 128, NCH], bf16,
                         kind="ExternalOutput").ap()

    CHUNKS = [(0, 512), (512, NCH - 512)]  # (n0, nn)

    with ExitStack() as ctx:
        tc = ctx.enter_context(tile.TileContext(nc))
        wpool = ctx.enter_context(tc.tile_pool(name="wp", bufs=1))
        bpool = ctx.enter_context(tc.tile_pool(name="bp", bufs=1))
        xpool = ctx.enter_context(tc.tile_pool(name="xp", bufs=4))
        opool = ctx.enter_context(tc.tile_pool(name="op", bufs=3))
        pspool = ctx.enter_context(tc.tile_pool(name="ps", bufs=3,
                                                space="PSUM"))
        wmpool = ctx.enter_context(tc.tile_pool(name="wm", bufs=1,
                                                space="PSUM"))

        # warm the HAM clock gate during the DMA-bound startup window:
        # tiny matmuls keep PE busy so the 2.4 GHz gate is open when the
        # real stream starts
        z1 = wpool.tile([128, 1], bf16, tag="z1")
        nc.gpsimd.memset(z1[:], 0.0)
        wps = wmpool.tile([8, 256], f32, tag="warm")
        for _ in range(40):
            nc.tensor.matmul(wps[:1, :1], z1[:, :1], z1[:, :1],
                             start=True, stop=True, skip_group_check=True)
        scr = wpool.tile([128, 256], bf16, tag="scr")
        nc.gpsimd.memset(scr[:], 0.0)
        for _ in range(40):
            nc.tensor.matmul(wps[:, :], scr[:, :8], scr[:, :],
                             start=True, stop=True, skip_group_check=True)

        b_t = bpool.tile([128, 1], f32)
        nc.gpsimd.dma_start(b_t[:], b_d[:])
        wt_t = wpool.tile([KL, NPASS * 128], bf16)
        nc.scalar.dma_start(wt_t[:, :], w_d[:, :])

        for s in range(NSETS):
            ld = nc.sync if s % 2 == 0 else nc.scalar
            st = nc.scalar if s % 2 == 0 else nc.sync
            x_t = xpool.tile([KL, TW2], bf16)
            ld.dma_start(x_t[:, :], x_d[s, :, :])
            ps_a = pspool.tile([128, 512], f32, tag="ps0")
            ps_b = pspool.tile([128, 512], f32, tag="ps1")
            pss = [ps_a, ps_b]
            for p in range(NPASS):
                for ci, (n0, nn) in enumerate(CHUNKS):
                    c0 = p + 2 * n0
                    nc.tensor.matmul(
                        pss[ci][:, :nn],
                        wt_t[:, p * 128:(p + 1) * 128],
                        x_t[:, c0: c0 + 2 * nn: 2],
                        start=(p == 0),
                        stop=(p == NPASS - 1),
                    )
                if p in (4, NPASS - 1):
                    # strided-rhs matmuls don't register as PE-busy in the
                    # HAM activity monitor (the clock gate never opens and
                    # everything streams at 1.2 GHz).  A tiny contiguous-rhs
                    # matmul every ~2us keeps the 2.4 GHz gate open; using
                    # x_t as input makes Tile schedule it inside this set.
                    nc.tensor.matmul(wps[:8, :128], x_t[:, :8],
                                     x_t[:, 0:128], start=True, stop=True,
                                     skip_group_check=True)
            o_t = opool.tile([128, NCH], bf16)
            for ci, (n0, nn) in enumerate(CHUNKS):
                nc.vector.tensor_scalar_add(o_t[:, n0:n0 + nn],
                                            pss[ci][:, :nn], b_t[:])
            st.dma_start(o_d[s, :, :], o_t[:, :])

    nc.compile()
    return nc


def _build_weights(weight):
    wt = np.zeros((KL, NPASS * 128), np.float32)
    for p in range(NPASS):
        for cp in range(2):
            for bw in range(BW):
                kw = 2 * p + cp - bw
                if not (0 <= kw < KW):
                    continue
                for bh in range(BH):
                    rp = np.arange(KH) + bh   # lanes rp = bh + kh
                    wt[cp * R + rp, p * 128 + bw * BH + bh] = weight[:, kw]
    return wt


def _prepare_in_maps(x, weight, bias):
    import ml_dtypes
    x = np.asarray(x, dtype=np.float32)
    weight = np.asarray(weight, dtype=np.float32)
    bias = np.asarray(bias, dtype=np.float32)

    x_pad = np.zeros((NCORES * CROWS + KH - 1, 2 * TW2), np.float32)
    x_pad[:H, :W] = x
    x_bf = x_pad.astype(ml_dtypes.bfloat16)
    wt = _build_weights(weight).astype(ml_dtypes.bfloat16)
    bias_b = np.full((128, 1), bias.reshape(-1)[0], np.float32)

    # pack per-set lane tiles [NSETS, 92, TW2]:
    # lanes 0..45 = even cols of rows r0..r0+45, lanes 46..91 = odd cols
    in_maps = []
    for c in range(NCORES):
        xc = x_bf[c * CROWS: c * CROWS + XROWS, :]
        xab = np.empty((NSETS, KL, TW2), dtype=ml_dtypes.bfloat16)
        for s in range(NSETS):
            rows = xc[s * BH: s * BH + R, :]
            xab[s, 0:R, :] = rows[:, 0::2]
            xab[s, R:KL, :] = rows[:, 1::2]
        in_maps.append({"x": xab, "wt": wt, "bias": bias_b})
    return in_maps


def _run(x, weight, bias, trace=False):
    from concourse.bass_utils import run_bass_kernel_spmd

    if "nc" not in _CACHE:
        _CACHE["nc"] = _build_program()
    nc = _CACHE["nc"]

    in_maps = _prepare_in_maps(x, weight, bias)
    res = run_bass_kernel_spmd(nc, in_maps, core_ids=list(range(NCORES)),
                               trace=trace)
    out = np.empty((NCORES * CROWS, BW * NCH), np.float32)
    for c in range(NCORES):
        blk = np.asarray(res.results[c]["out"], dtype=np.float32)
        # blk [NSETS, 128=(bw,bh), NCH] -> [rows, 4n+bw]
        out[c * CROWS:(c + 1) * CROWS, :] = (
            blk.reshape(NSETS, BW, BH, NCH)
               .transpose(0, 2, 3, 1)
               .reshape(CROWS, BW * NCH))
    return out[:OH, :OW], res


def kernel(x, weight, bias):
    out, _ = _run(x, weight, bias, trace=False)
    return out


# revision 25
# speedup vs baseline: 1.0663x; 1.0663x over previous
"""Trainium2 Bass kernel: 15x15 valid cross-correlation of 4096x4096 (+bias).

Sharding: output rows split across 8 NeuronCores (512 rows/core, 14-row
halo), gathered on the host.

Per core the conv runs as a 2-shift im2col-lite on the TensorEngine: the
K-lanes hold two column-shifted views of a 46-row x slab (partitions
0..45 = shift 0, 46..91 = shift +1 col) and each pass's stationary matrix
maps lane (shift c', row r') to output (col-phase bw in 0..3, row bh in
0..31):

    T_p[c'*46+r', bw*32+bh] = w[r' - bh, 2p + c' - bw]    p = 0..8

so one [92 x 128] x [92 x N] matmul covers 2 kernel columns for a
32-row x 4-col-phase output block (M=128 full): 9 passes/32 rows vs the
banded-Toeplitz 15/114 (147k vs 276k PE cycles/core).

Key DMA tricks (each measured as a cliff on HW):
- shift-0 lanes are only read at even cols (2p+4n), shift-1 at odd, so
  the host packs even cols for lanes 0..45 and odd cols for 46..91 into
  one contiguous [NSETS, 92, 2050] tensor: zero duplication (6 MB/core)
  and the rhs is a uniform stride-2 (8B) AP = full-rate SBUF stream.
- every dma_start must be one contiguous HBM range: a strided read
  chains all descriptors onto a single SDMA engine (~27 GiB/s).
- each HWDGE queue (sync/scalar) only drives ~4 SDMA engines, and a
  queue is FIFO: a store waiting on a drain blocks later loads queued
  behind it.  Loads and stores alternate queues by set parity instead.
- output drains as one contiguous [128, 1021] block per set (psum
  partition = bw*32+bh, col n <-> out col 4n+bw); host re-interleaves.

Weights/x in bf16 (fp32 PSUM accumulate), same numerics as the reference
Toeplitz baseline (~3e-3 rel err).
"""

import numpy as np

H = 4096
W = 4096
KH = 15
KW = 15
OH = H - KH + 1  # 4082
OW = W - KW + 1  # 4082
NCORES = 8
CROWS = 512             # output rows per core
BH = 32                 # output rows per set
BW = 4                  # output col-phases
NSETS = CROWS // BH     # 16
NPASS = 9               # ceil((BW + KW - 1) / 2)
R = BH + KH - 1         # 46 rows per shift-copy
KL = 2 * R              # 92 K-lanes
NCH = (OW + BW - 1) // BW       # 1021 psum cols (4*1021=4084 >= 4082)
QW = 1026               # packed quarter-width (max read idx (p>>1)+n = 1024)
XROWS = CROWS + KH - 1  # 526

_CACHE = {}


def _build_program():
    import concourse.tile as tile
    from concourse import bacc, mybir
    from contextlib import ExitStack

    nc = bacc.Bacc("TRN2", target_bir_lowering=False, debug=False,
                   num_devices=NCORES)
    bf16 = mybir.dt.bfloat16
    f32 = mybir.dt.float32
    x_d = nc.dram_tensor("x", [NSETS, KL, 2 * QW], bf16,
                         kind="ExternalInput").ap()
    w_d = nc.dram_tensor("wt", [KL, NPASS * 128], bf16,
                         kind="ExternalInput").ap()
    b_d = nc.dram_tensor("bias", [128, 1], f32, kind="ExternalInput").ap()
    o_d = nc.dram_tensor("out", [NSETS, 128, NCH], bf16,
                         kind="ExternalOutput").ap()

    CHUNKS = [(0, 512), (512, NCH - 512)]  # (n0, nn)

    with ExitStack() as ctx:
        tc = ctx.enter_context(tile.TileContext(nc))
        wpool = ctx.enter_context(tc.tile_pool(name="wp", bufs=1))
        bpool = ctx.enter_context(tc.tile_pool(name="bp", bufs=1))
        xpool = ctx.enter_context(tc.tile_pool(name="xp", bufs=4))
        opool = ctx.enter_context(tc.tile_pool(name="op", bufs=3))
        pspool = ctx.enter_context(tc.tile_pool(name="ps", bufs=3,
                                                space="PSUM"))
        wmpool = ctx.enter_context(tc.tile_pool(name="wm", bufs=1,
                                                space="PSUM"))

        # warm the HAM clock gate during the DMA-bound startup window:
        # tiny matmuls keep PE busy so the 2.4 GHz gate is open when the
        # real stream starts
        z1 = wpool.tile([128, 1], bf16, tag="z1")
        nc.gpsimd.memset(z1[:], 0.0)
        wps = wmpool.tile([8, 256], f32, tag="warm")
        for _ in range(40):
            nc.tensor.matmul(wps[:1, :1], z1[:, :1], z1[:, :1],
                             start=True, stop=True, skip_group_check=True)
        scr = wpool.tile([128, 256], bf16, tag="scr")
        nc.gpsimd.memset(scr[:], 0.0)
        for _ in range(40):
            nc.tensor.matmul(wps[:, :], scr[:, :8], scr[:, :],
                             start=True, stop=True, skip_group_check=True)

        b_t = bpool.tile([128, 1], f32)
        nc.gpsimd.dma_start(b_t[:], b_d[:])
        wt_t = wpool.tile([KL, NPASS * 128], bf16)
        nc.scalar.dma_start(wt_t[:, :], w_d[:, :])

        for s in range(NSETS):
            ld = nc.sync if s % 2 == 0 else nc.scalar
            st = nc.scalar if s % 2 == 0 else nc.sync
            x_t = xpool.tile([KL, 2 * QW], bf16)
            ld.dma_start(x_t[:, :], x_d[s, :, :])
            ps_a = pspool.tile([128, 512], f32, tag="ps0")
            ps_b = pspool.tile([128, 512], f32, tag="ps1")
            pss = [ps_a, ps_b]
            for p in range(NPASS):
                # quarter-plane layout keeps every rhs slice contiguous:
                # strided-rhs matmuls never register as PE-busy in the HAM
                # activity monitor, locking the clock gate at 1.2 GHz
                b = (p >> 1) + (p & 1) * QW
                for ci, (n0, nn) in enumerate(CHUNKS):
                    nc.tensor.matmul(
                        pss[ci][:, :nn],
                        wt_t[:, p * 128:(p + 1) * 128],
                        x_t[:, b + n0: b + n0 + nn],
                        start=(p == 0),
                        stop=(p == NPASS - 1),
                    )
            o_t = opool.tile([128, NCH], bf16)
            for ci, (n0, nn) in enumerate(CHUNKS):
                nc.vector.tensor_scalar_add(o_t[:, n0:n0 + nn],
                                            pss[ci][:, :nn], b_t[:])
            st.dma_start(o_d[s, :, :], o_t[:, :])

    nc.compile()
    return nc


def _build_weights(weight):
    wt = np.zeros((KL, NPASS * 128), np.float32)
    for p in range(NPASS):
        for cp in range(2):
            for bw in range(BW):
                kw = 2 * p + cp - bw
                if not (0 <= kw < KW):
                    continue
                for bh in range(BH):
                    rp = np.arange(KH) + bh   # lanes rp = bh + kh
                    wt[cp * R + rp, p * 128 + bw * BH + bh] = weight[:, kw]
    return wt


def _prepare_in_maps(x, weight, bias):
    import ml_dtypes
    x = np.asarray(x, dtype=np.float32)
    weight = np.asarray(weight, dtype=np.float32)
    bias = np.asarray(bias, dtype=np.float32)

    x_pad = np.zeros((NCORES * CROWS + KH - 1, 4 * QW), np.float32)
    x_pad[:H, :W] = x
    x_bf = x_pad.astype(ml_dtypes.bfloat16)
    wt = _build_weights(weight).astype(ml_dtypes.bfloat16)
    bias_b = np.full((128, 1), bias.reshape(-1)[0], np.float32)

    # pack per-set lane tiles [NSETS, 92, 2*QW] as 4 quarter-planes:
    # lanes 0..45 (shift 0) = [cols 0 mod 4 | cols 2 mod 4],
    # lanes 46..91 (shift 1) = [cols 1 mod 4 | cols 3 mod 4];
    # pass p reads the contiguous range (p>>1) + (p&1)*QW + n
    in_maps = []
    for c in range(NCORES):
        xc = x_bf[c * CROWS: c * CROWS + XROWS, :]
        xab = np.empty((NSETS, KL, 2 * QW), dtype=ml_dtypes.bfloat16)
        for s in range(NSETS):
            rows = xc[s * BH: s * BH + R, :]
            xab[s, 0:R, 0:QW] = rows[:, 0::4]
            xab[s, 0:R, QW:] = rows[:, 2::4]
            xab[s, R:KL, 0:QW] = rows[:, 1::4]
            xab[s, R:KL, QW:] = rows[:, 3::4]
        in_maps.append({"x": xab, "wt": wt, "bias": bias_b})
    return in_maps


def _run(x, weight, bias, trace=False):
    from concourse.bass_utils import run_bass_kernel_spmd

    if "nc" not in _CACHE:
        _CACHE["nc"] = _build_program()
    nc = _CACHE["nc"]

    in_maps = _prepare_in_maps(x, weight, bias)
    res = run_bass_kernel_spmd(nc, in_maps, core_ids=list(range(NCORES)),
                               trace=trace)
    out = np.empty((NCORES * CROWS, BW * NCH), np.float32)
    for c in range(NCORES):
        blk = np.asarray(res.results[c]["out"], dtype=np.float32)
        # blk [NSETS, 128=(bw,bh), NCH] -> [rows, 4n+bw]
        out[c * CROWS:(c + 1) * CROWS, :] = (
            blk.reshape(NSETS, BW, BH, NCH)
               .transpose(0, 2, 3, 1)
               .reshape(CROWS, BW * NCH))
    return out[:OH, :OW], res


def kernel(x, weight, bias):
    out, _ = _run(x, weight, bias, trace=False)
    return out


# revision 26
# speedup vs baseline: 1.6615x; 1.5582x over previous
"""Trainium2 Bass kernel: 15x15 valid cross-correlation of 4096x4096 (+bias).

Sharding: output rows split across 8 NeuronCores (512 rows/core, 14-row
halo), gathered on the host.

Per core the conv runs as a 2-shift im2col-lite on the TensorEngine: the
K-lanes hold two column-shifted views of a 46-row x slab (partitions
0..45 = shift 0, 46..91 = shift +1 col) and each pass's stationary matrix
maps lane (shift c', row r') to output (col-phase bw in 0..3, row bh in
0..31):

    T_p[c'*46+r', bw*32+bh] = w[r' - bh, 2p + c' - bw]    p = 0..8

so one [92 x 128] x [92 x N] matmul covers 2 kernel columns for a
32-row x 4-col-phase output block (M=128 full): 9 passes/32 rows vs the
banded-Toeplitz 15/114 (147k vs 276k PE cycles/core).

Key DMA tricks (each measured as a cliff on HW):
- shift-0 lanes are only read at even cols (2p+4n), shift-1 at odd, so
  the host packs even cols for lanes 0..45 and odd cols for 46..91 into
  one contiguous [NSETS, 92, 2050] tensor: zero duplication (6 MB/core)
  and the rhs is a uniform stride-2 (8B) AP = full-rate SBUF stream.
- every dma_start must be one contiguous HBM range: a strided read
  chains all descriptors onto a single SDMA engine (~27 GiB/s).
- each HWDGE queue (sync/scalar) only drives ~4 SDMA engines, and a
  queue is FIFO: a store waiting on a drain blocks later loads queued
  behind it.  Loads and stores alternate queues by set parity instead.
- output drains as one contiguous [128, 1021] block per set (psum
  partition = bw*32+bh, col n <-> out col 4n+bw); host re-interleaves.

Weights/x in bf16 (fp32 PSUM accumulate), same numerics as the reference
Toeplitz baseline (~3e-3 rel err).
"""

import numpy as np

H = 4096
W = 4096
KH = 15
KW = 15
OH = H - KH + 1  # 4082
OW = W - KW + 1  # 4082
NCORES = 8
CROWS = 512             # output rows per core
BH = 32                 # output rows per set
BW = 4                  # output col-phases
NSETS = CROWS // BH     # 16
NPASS = 9               # ceil((BW + KW - 1) / 2)
R = BH + KH - 1         # 46 rows per shift-copy
KL = 2 * R              # 92 live K-lanes
KP = 128                # padded K (FWL needs NumWeights==128; rows 92..127
                        # of the stationary are zero so lanes 92..127 of the
                        # rhs may stream garbage)
NCH = (OW + BW - 1) // BW       # 1021 psum cols (4*1021=4084 >= 4082)
QW = 1026               # packed quarter-width (max read idx (p>>1)+n = 1024)
XROWS = CROWS + KH - 1  # 526

_CACHE = {}


def _build_program():
    import concourse.tile as tile
    from concourse import bacc, mybir
    from contextlib import ExitStack

    nc = bacc.Bacc("TRN2", target_bir_lowering=False, debug=False,
                   num_devices=NCORES)
    bf16 = mybir.dt.bfloat16
    f32 = mybir.dt.float32
    x_d = nc.dram_tensor("x", [NSETS, KL, 2 * QW], bf16,
                         kind="ExternalInput").ap()
    w_d = nc.dram_tensor("wt", [KP, NPASS * 128], bf16,
                         kind="ExternalInput").ap()
    b_d = nc.dram_tensor("bias", [128, 1], f32, kind="ExternalInput").ap()
    o_d = nc.dram_tensor("out", [NSETS, 128, NCH], bf16,
                         kind="ExternalOutput").ap()

    CHUNKS = [(0, 512), (512, NCH - 512)]  # (n0, nn)

    with ExitStack() as ctx:
        tc = ctx.enter_context(tile.TileContext(nc))
        wpool = ctx.enter_context(tc.tile_pool(name="wp", bufs=1))
        bpool = ctx.enter_context(tc.tile_pool(name="bp", bufs=1))
        xpool = ctx.enter_context(tc.tile_pool(name="xp", bufs=4))
        opool = ctx.enter_context(tc.tile_pool(name="op", bufs=3))
        pspool = ctx.enter_context(tc.tile_pool(name="ps", bufs=3,
                                                space="PSUM"))
        wmpool = ctx.enter_context(tc.tile_pool(name="wm", bufs=1,
                                                space="PSUM"))

        # warm the HAM clock gate during the DMA-bound startup window:
        # tiny matmuls keep PE busy so the 2.4 GHz gate is open when the
        # real stream starts
        z1 = wpool.tile([128, 1], bf16, tag="z1")
        nc.gpsimd.memset(z1[:], 0.0)
        wps = wmpool.tile([8, 256], f32, tag="warm")
        for _ in range(40):
            nc.tensor.matmul(wps[:1, :1], z1[:, :1], z1[:, :1],
                             start=True, stop=True, skip_group_check=True)
        scr = wpool.tile([128, 256], bf16, tag="scr")
        nc.gpsimd.memset(scr[:], 0.0)
        for _ in range(40):
            nc.tensor.matmul(wps[:, :], scr[:, :8], scr[:, :],
                             start=True, stop=True, skip_group_check=True)

        b_t = bpool.tile([128, 1], f32)
        nc.gpsimd.dma_start(b_t[:], b_d[:])
        wt_t = wpool.tile([KP, NPASS * 128], bf16)
        nc.scalar.dma_start(wt_t[:, :], w_d[:, :])

        for s in range(NSETS):
            ld = nc.sync if s % 2 == 0 else nc.scalar
            st = nc.scalar if s % 2 == 0 else nc.sync
            x_t = xpool.tile([KP, 2 * QW], bf16)
            ld.dma_start(x_t[0:KL, :], x_d[s, :, :])
            ps_a = pspool.tile([128, 512], f32, tag="ps0")
            ps_b = pspool.tile([128, 512], f32, tag="ps1")
            pss = [ps_a, ps_b]
            for p in range(NPASS):
                # quarter-plane layout keeps every rhs slice contiguous:
                # strided-rhs matmuls never register as PE-busy in the HAM
                # activity monitor, locking the clock gate at 1.2 GHz
                b = (p >> 1) + (p & 1) * QW
                for ci, (n0, nn) in enumerate(CHUNKS):
                    nc.tensor.matmul(
                        pss[ci][:, :nn],
                        wt_t[:, p * 128:(p + 1) * 128],
                        x_t[:, b + n0: b + n0 + nn],
                        start=(p == 0),
                        stop=(p == NPASS - 1),
                    )
            o_t = opool.tile([128, NCH], bf16)
            for ci, (n0, nn) in enumerate(CHUNKS):
                nc.vector.tensor_scalar_add(o_t[:, n0:n0 + nn],
                                            pss[ci][:, :nn], b_t[:])
            st.dma_start(o_d[s, :, :], o_t[:, :])

    nc.compile()
    return nc


def _build_weights(weight):
    wt = np.zeros((KP, NPASS * 128), np.float32)
    for p in range(NPASS):
        for cp in range(2):
            for bw in range(BW):
                kw = 2 * p + cp - bw
                if not (0 <= kw < KW):
                    continue
                for bh in range(BH):
                    rp = np.arange(KH) + bh   # lanes rp = bh + kh
                    wt[cp * R + rp, p * 128 + bw * BH + bh] = weight[:, kw]
    return wt


def _prepare_in_maps(x, weight, bias):
    import ml_dtypes
    x = np.asarray(x, dtype=np.float32)
    weight = np.asarray(weight, dtype=np.float32)
    bias = np.asarray(bias, dtype=np.float32)

    x_pad = np.zeros((NCORES * CROWS + KH - 1, 4 * QW), np.float32)
    x_pad[:H, :W] = x
    x_bf = x_pad.astype(ml_dtypes.bfloat16)
    wt = _build_weights(weight).astype(ml_dtypes.bfloat16)
    bias_b = np.full((128, 1), bias.reshape(-1)[0], np.float32)

    # pack per-set lane tiles [NSETS, 92, 2*QW] as 4 quarter-planes:
    # lanes 0..45 (shift 0) = [cols 0 mod 4 | cols 2 mod 4],
    # lanes 46..91 (shift 1) = [cols 1 mod 4 | cols 3 mod 4];
    # pass p reads the contiguous range (p>>1) + (p&1)*QW + n
    in_maps = []
    for c in range(NCORES):
        xc = x_bf[c * CROWS: c * CROWS + XROWS, :]
        xab = np.empty((NSETS, KL, 2 * QW), dtype=ml_dtypes.bfloat16)
        for s in range(NSETS):
            rows = xc[s * BH: s * BH + R, :]
            xab[s, 0:R, 0:QW] = rows[:, 0::4]
            xab[s, 0:R, QW:] = rows[:, 2::4]
            xab[s, R:KL, 0:QW] = rows[:, 1::4]
            xab[s, R:KL, QW:] = rows[:, 3::4]
        in_maps.append({"x": xab, "wt": wt, "bias": bias_b})
    return in_maps


def _run(x, weight, bias, trace=False):
    from concourse.bass_utils import run_bass_kernel_spmd

    if "nc" not in _CACHE:
        _CACHE["nc"] = _build_program()
    nc = _CACHE["nc"]

    in_maps = _prepare_in_maps(x, weight, bias)
    res = run_bass_kernel_spmd(nc, in_maps, core_ids=list(range(NCORES)),
                               trace=trace)
    out = np.empty((NCORES * CROWS, BW * NCH), np.float32)
    for c in range(NCORES):
        blk = np.asarray(res.results[c]["out"], dtype=np.float32)
        # blk [NSETS, 128=(bw,bh), NCH] -> [rows, 4n+bw]
        out[c * CROWS:(c + 1) * CROWS, :] = (
            blk.reshape(NSETS, BW, BH, NCH)
               .transpose(0, 2, 3, 1)
               .reshape(CROWS, BW * NCH))
    return out[:OH, :OW], res


def kernel(x, weight, bias):
    out, _ = _run(x, weight, bias, trace=False)
    return out


# revision 29
# speedup vs baseline: 1.7145x; 1.0319x over previous
"""Trainium2 Bass kernel: 15x15 valid cross-correlation of 4096x4096 (+bias).

Sharding: output rows split across 8 NeuronCores (512 rows/core, 14-row
halo), gathered on the host.

Per core the conv runs as a 2-shift im2col-lite on the TensorEngine: the
K-lanes hold two column-shifted views of a 46-row x slab (partitions
0..45 = shift 0, 46..91 = shift +1 col) and each pass's stationary matrix
maps lane (shift c', row r') to output (col-phase bw in 0..3, row bh in
0..31):

    T_p[c'*46+r', bw*32+bh] = w[r' - bh, 2p + c' - bw]    p = 0..8

so one [92 x 128] x [92 x N] matmul covers 2 kernel columns for a
32-row x 4-col-phase output block (M=128 full): 9 passes/32 rows vs the
banded-Toeplitz 15/114 (147k vs 276k PE cycles/core).

Key DMA tricks (each measured as a cliff on HW):
- shift-0 lanes are only read at even cols (2p+4n), shift-1 at odd, so
  the host packs even cols for lanes 0..45 and odd cols for 46..91 into
  one contiguous [NSETS, 92, 2050] tensor: zero duplication (6 MB/core)
  and the rhs is a uniform stride-2 (8B) AP = full-rate SBUF stream.
- every dma_start must be one contiguous HBM range: a strided read
  chains all descriptors onto a single SDMA engine (~27 GiB/s).
- each HWDGE queue (sync/scalar) only drives ~4 SDMA engines, and a
  queue is FIFO: a store waiting on a drain blocks later loads queued
  behind it.  Loads and stores alternate queues by set parity instead.
- output drains as one contiguous [128, 1021] block per set (psum
  partition = bw*32+bh, col n <-> out col 4n+bw); host re-interleaves.

Weights/x in bf16 (fp32 PSUM accumulate), same numerics as the reference
Toeplitz baseline (~3e-3 rel err).
"""

import numpy as np

H = 4096
W = 4096
KH = 15
KW = 15
OH = H - KH + 1  # 4082
OW = W - KW + 1  # 4082
NCORES = 8
CROWS = 512             # output rows per core
BH = 32                 # output rows per set
BW = 4                  # output col-phases
NSETS = CROWS // BH     # 16
NPASS = 9               # ceil((BW + KW - 1) / 2)
R = BH + KH - 1         # 46 rows per shift-copy
KL = 2 * R              # 92 live K-lanes
KP = 128                # padded K (FWL needs NumWeights==128; rows 92..127
                        # of the stationary are zero so lanes 92..127 of the
                        # rhs may stream garbage)
NCH = (OW + BW - 1) // BW       # 1021 psum cols (4*1021=4084 >= 4082)
QW = 1026               # packed quarter-width (max read idx (p>>1)+n = 1024)
XROWS = CROWS + KH - 1  # 526

_CACHE = {}


def _build_program():
    import concourse.tile as tile
    from concourse import bacc, mybir
    from contextlib import ExitStack

    nc = bacc.Bacc("TRN2", target_bir_lowering=False, debug=False,
                   num_devices=NCORES)
    bf16 = mybir.dt.bfloat16
    f32 = mybir.dt.float32
    x_d = nc.dram_tensor("x", [NSETS, KL, 2 * QW], bf16,
                         kind="ExternalInput").ap()
    w_d = nc.dram_tensor("wt", [KP, NPASS * 128], bf16,
                         kind="ExternalInput").ap()
    b_d = nc.dram_tensor("bias", [128, 1], f32, kind="ExternalInput").ap()
    o_d = nc.dram_tensor("out", [NSETS, 128, NCH], bf16,
                         kind="ExternalOutput").ap()

    CHUNKS = [(0, 512), (512, NCH - 512)]  # (n0, nn)

    with ExitStack() as ctx:
        tc = ctx.enter_context(tile.TileContext(nc))
        wpool = ctx.enter_context(tc.tile_pool(name="wp", bufs=1))
        bpool = ctx.enter_context(tc.tile_pool(name="bp", bufs=1))
        xpool = ctx.enter_context(tc.tile_pool(name="xp", bufs=5))
        opool = ctx.enter_context(tc.tile_pool(name="op", bufs=3))
        pspool = ctx.enter_context(tc.tile_pool(name="ps", bufs=3,
                                                space="PSUM"))
        wmpool = ctx.enter_context(tc.tile_pool(name="wm", bufs=1,
                                                space="PSUM"))

        # warm the HAM clock gate during the DMA-bound startup window:
        # tiny matmuls keep PE busy so the 2.4 GHz gate is open when the
        # real stream starts
        z1 = wpool.tile([128, 1], bf16, tag="z1")
        nc.gpsimd.memset(z1[:], 0.0)
        wps = wmpool.tile([8, 256], f32, tag="warm")
        for _ in range(40):
            nc.tensor.matmul(wps[:1, :1], z1[:, :1], z1[:, :1],
                             start=True, stop=True, skip_group_check=True)
        scr = wpool.tile([128, 256], bf16, tag="scr")
        nc.gpsimd.memset(scr[:], 0.0)
        for _ in range(40):
            nc.tensor.matmul(wps[:, :], scr[:, :8], scr[:, :],
                             start=True, stop=True, skip_group_check=True)

        b_t = bpool.tile([128, 1], f32)
        nc.gpsimd.dma_start(b_t[:], b_d[:])
        wt_t = wpool.tile([KP, NPASS * 128], bf16)
        nc.scalar.dma_start(wt_t[:, :], w_d[:, :])

        # loads are issued LOOKAHEAD sets early so a store (which waits on
        # its drain) queued on the same FIFO queue never delays a load
        LOOKAHEAD = 4
        x_tiles = {}

        def issue_load(s):
            if s >= NSETS:
                return
            ld = nc.sync if s % 2 == 0 else nc.scalar
            x_t = xpool.tile([KP, 2 * QW], bf16)
            ld.dma_start(x_t[0:KL, :], x_d[s, :, :])
            x_tiles[s] = x_t

        for s in range(LOOKAHEAD):
            issue_load(s)

        for s in range(NSETS):
            st = nc.scalar if s % 2 == 0 else nc.sync
            x_t = x_tiles.pop(s)
            ps_a = pspool.tile([128, 512], f32, tag="ps0")
            ps_b = pspool.tile([128, 512], f32, tag="ps1")
            pss = [ps_a, ps_b]
            for p in range(NPASS):
                # quarter-plane layout keeps every rhs slice contiguous:
                # strided-rhs matmuls never register as PE-busy in the HAM
                # activity monitor, locking the clock gate at 1.2 GHz
                b = (p >> 1) + (p & 1) * QW
                for ci, (n0, nn) in enumerate(CHUNKS):
                    nc.tensor.matmul(
                        pss[ci][:, :nn],
                        wt_t[:, p * 128:(p + 1) * 128],
                        x_t[:, b + n0: b + n0 + nn],
                        start=(p == 0),
                        stop=(p == NPASS - 1),
                    )
            issue_load(s + LOOKAHEAD)
            o_t = opool.tile([128, NCH], bf16)
            for ci, (n0, nn) in enumerate(CHUNKS):
                nc.vector.tensor_scalar_add(o_t[:, n0:n0 + nn],
                                            pss[ci][:, :nn], b_t[:])
            st.dma_start(o_d[s, :, :], o_t[:, :])

    nc.compile()
    return nc


def _build_weights(weight):
    wt = np.zeros((KP, NPASS * 128), np.float32)
    for p in range(NPASS):
        for cp in range(2):
            for bw in range(BW):
                kw = 2 * p + cp - bw
                if not (0 <= kw < KW):
                    continue
                for bh in range(BH):
                    rp = np.arange(KH) + bh   # lanes rp = bh + kh
                    wt[cp * R + rp, p * 128 + bw * BH + bh] = weight[:, kw]
    return wt


def _prepare_in_maps(x, weight, bias):
    import ml_dtypes
    x = np.asarray(x, dtype=np.float32)
    weight = np.asarray(weight, dtype=np.float32)
    bias = np.asarray(bias, dtype=np.float32)

    x_pad = np.zeros((NCORES * CROWS + KH - 1, 4 * QW), np.float32)
    x_pad[:H, :W] = x
    x_bf = x_pad.astype(ml_dtypes.bfloat16)
    wt = _build_weights(weight).astype(ml_dtypes.bfloat16)
    bias_b = np.full((128, 1), bias.reshape(-1)[0], np.float32)

    # pack per-set lane tiles [NSETS, 92, 2*QW] as 4 quarter-planes:
    # lanes 0..45 (shift 0) = [cols 0 mod 4 | cols 2 mod 4],
    # lanes 46..91 (shift 1) = [cols 1 mod 4 | cols 3 mod 4];
    # pass p reads the contiguous range (p>>1) + (p&1)*QW + n
    in_maps = []
    for c in range(NCORES):
        xc = x_bf[c * CROWS: c * CROWS + XROWS, :]
        xab = np.empty((NSETS, KL, 2 * QW), dtype=ml_dtypes.bfloat16)
        for s in range(NSETS):
            rows = xc[s * BH: s * BH + R, :]
            xab[s, 0:R, 0:QW] = rows[:, 0::4]
            xab[s, 0:R, QW:] = rows[:, 2::4]
            xab[s, R:KL, 0:QW] = rows[:, 1::4]
            xab[s, R:KL, QW:] = rows[:, 3::4]
        in_maps.append({"x": xab, "wt": wt, "bias": bias_b})
    return in_maps


def _run(x, weight, bias, trace=False):
    from concourse.bass_utils import run_bass_kernel_spmd

    if "nc" not in _CACHE:
        _CACHE["nc"] = _build_program()
    nc = _CACHE["nc"]

    in_maps = _prepare_in_maps(x, weight, bias)
    res = run_bass_kernel_spmd(nc, in_maps, core_ids=list(range(NCORES)),
                               trace=trace)
    out = np.empty((NCORES * CROWS, BW * NCH), np.float32)
    for c in range(NCORES):
        blk = np.asarray(res.results[c]["out"], dtype=np.float32)
        # blk [NSETS, 128=(bw,bh), NCH] -> [rows, 4n+bw]
        out[c * CROWS:(c + 1) * CROWS, :] = (
            blk.reshape(NSETS, BW, BH, NCH)
               .transpose(0, 2, 3, 1)
               .reshape(CROWS, BW * NCH))
    return out[:OH, :OW], res


def kernel(x, weight, bias):
    out, _ = _run(x, weight, bias, trace=False)
    return out
